# revision 39
# baseline (speedup 1.0000x reference)
"""Multi-head attention (B=4, N=2048, C=1024, H=16, D=64) on 8 TRN2 NeuronCores.

Sharding: data-parallel over batch (4) x tensor-parallel over head halves (2).
Core c handles batch b=c//2, head-group g=c%2 (8 heads). Each core computes a
partial output (its heads' contribution to the out-projection, pre-bias);
the host sums the two partials per batch and adds the bias.

Per-core device graph (matmul inputs bf16, psum f32):
  QT/KT = (wq/wk slice)@x^T in transposed layout [o_local=512, n=2048]
  V     = x @ wv^T in normal layout [n, o_local], with a ones-column per head
  scoresT[j,i] per head pair via row-packed K=64 matmuls
  expT = exp(SCALE * scoresT)  (ACT engine, one op per 2 psum banks)
  PV: psum[65, i] = [V_h | ones]^T @ expT -> rows 0-63 = O'^T, row 64 = l_i
      (software-pipelined one jt behind the scores matmuls)
  normalize: batched 1/l = exp(-ln(l)) on the ACT engine (rows {0,32,64,96}
      of an l-stash tile), partition-broadcast via K=1 bf16 outer products
      into one shared PSUM bank per head pair, in-place multiply on O^T
  out-proj: outT[o, n] = woT^T @ O^T (partial, bf16 out; host sums + bias)

PV matmuls run two jt behind the scores matmuls (software pipeline), and
projection matmuls for phase p+1 plus phase p-1's slot-gated epilogue are
interleaved into the attention stream ("fillers") so the strictly-FIFO PE
queue never sits behind a dependency-stalled instruction.
"""
import sys

sys.path.insert(0, "/opt/trn_rl_repo")
import numpy as np
import ml_dtypes
import bass_rust
import concourse.bass as bass
import concourse.mybir as mybir
import concourse.tile as tile
from concourse.bass_utils import run_bass_kernel_spmd

BF16 = mybir.dt.bfloat16
F32 = mybir.dt.float32
P = 128
B, N, C = 4, 2048, 1024
HG, D = 8, 64          # heads per core, head dim
OL = HG * D            # 512 local o dims
SCALE = D ** -0.5
NCT = C // P           # 8 contraction tiles
NOT = OL // P          # 4 o tiles (head pairs)
NNT = N // P           # 16 n tiles of 128
NNC = N // 512         # 4 n chunks of 512
VBLK = OL + HG         # 520: 8 heads x (64 V cols + 1 ones col)
RL_ROWS = (0, 32, 64, 96)  # legal matmul base partitions for the rl rhs


def _split_waits(nc, max_waits=1):
    """This walrus build rejects >1 sem-wait per instruction; split excess
    waits into preceding same-engine single-wait NOPs."""
    uid = [0]
    for f in nc.m.functions:
        for bb in f.blocks:
            out = []
            changed = False
            for inst in bb.instructions:
                si = inst.sync_info
                if si is not None and len(si.on_wait) > max_waits:
                    waits = list(si.on_wait)
                    for w in waits[:-max_waits]:
                        uid[0] += 1
                        nop = bass_rust.InstNoOp(name=f"I-waitsplit-{uid[0]}")
                        nop.engine = inst.engine
                        nop.sync_info = bass_rust.SyncInfo(on_wait=[w], on_update=[])
                        try:
                            nc.register_instruction(nop, overwrite=True)
                        except Exception:
                            pass
                        out.append(nop)
                    si.on_wait = waits[-max_waits:]
                    changed = True
                out.append(inst)
            if changed:
                bb.instructions = out


def build():
    nc = bass.Bass("TRN2", target_bir_lowering=False, debug=False)
    xT = nc.dram_tensor("xT", [C, N], BF16, kind="ExternalInput")
    wqT = nc.dram_tensor("wqT", [C, OL], BF16, kind="ExternalInput")
    wkT = nc.dram_tensor("wkT", [C, OL], BF16, kind="ExternalInput")
    wvT = nc.dram_tensor("wvT", [C, OL], BF16, kind="ExternalInput")
    woT = nc.dram_tensor("woT", [OL, C], BF16, kind="ExternalInput")
    outT = nc.dram_tensor("outT", [C, N], BF16, kind="ExternalOutput")

    with tile.TileContext(nc) as tc:
        with (
            tc.tile_pool(name="persist", bufs=1) as pp,
            tc.tile_pool(name="ex", bufs=4) as ep,
            tc.tile_pool(name="lrl", bufs=2) as lp,
            tc.tile_pool(name="ostage", bufs=8) as op,
            tc.tile_pool(name="proj_ps", bufs=2, space="PSUM") as proj_ps,
            tc.tile_pool(name="s_ps", bufs=2, space="PSUM") as s_ps,
            tc.tile_pool(name="pv_ps", bufs=1, space="PSUM") as pv_ps,
        ):
            # ---- persistent SBUF ----
            xt = [pp.tile([P, N], BF16, name=f"xt{i}", tag=f"xt{i}") for i in range(NCT)]
            wq_sb = pp.tile([P, NCT * OL], BF16, tag="wq")
            wk_sb = pp.tile([P, NCT * OL], BF16, tag="wk")
            wv_sb = pp.tile([P, NCT * OL], BF16, tag="wv")
            wo_sb = pp.tile([P, NOT * C], BF16, tag="wo")
            qt = [pp.tile([P, N], BF16, name=f"qt{p}", tag=f"qt{p}") for p in range(NOT)]
            kt = [pp.tile([P, N], BF16, name=f"kt{p}", tag=f"kt{p}") for p in range(NOT)]
            v_sb = pp.tile([P, NNT * VBLK], BF16, tag="v")
            ot_sb = [pp.tile([P, N], BF16, name=f"ot{p}", tag=f"ot{p}") for p in range(NOT)]
            ones_sb = pp.tile([97, D], BF16, tag="ones_sb")

            # ---- loads ----
            for i in range(NCT):
                nc.sync.dma_start(
                    out=wq_sb[:, i * OL:(i + 1) * OL],
                    in_=wqT[i * P:(i + 1) * P, :],
                )
                nc.sync.dma_start(
                    out=xt[i][:, 0:512], in_=xT[i * P:(i + 1) * P, 0:512])
            for i in range(NCT):
                nc.sync.dma_start(
                    out=wk_sb[:, i * OL:(i + 1) * OL],
                    in_=wkT[i * P:(i + 1) * P, :],
                )
            for nch in range(1, NNC):
                for i in range(NCT):
                    nc.sync.dma_start(
                        out=xt[i][:, nch * 512:(nch + 1) * 512],
                        in_=xT[i * P:(i + 1) * P, nch * 512:(nch + 1) * 512],
                    )
            for i in range(NCT):
                nc.sync.dma_start(
                    out=wv_sb[:, i * OL:(i + 1) * OL],
                    in_=wvT[i * P:(i + 1) * P, :],
                )
            for p_ in range(NOT):
                nc.sync.dma_start(
                    out=wo_sb[:, p_ * C:(p_ + 1) * C],
                    in_=woT[p_ * P:(p_ + 1) * P, :],
                )
            # ones columns for the softmax-denominator rows
            nc.any.memset(v_sb[:, :], 1.0)
            nc.any.memset(ones_sb[:, :], 1.0)

            def qk_proj_thunks(w_sb, dst, p_, knd):
                """Emit-callables for dst = transposed projection (Q or K)."""
                thunks = []
                for nch in range(NNC):
                    state = {}

                    def mk_mm(st, nch, ct):
                        def f():
                            if "ps" not in st:
                                st["ps"] = proj_ps.tile(
                                    [P, 512], F32, tag="proj",
                                    name=f"prj_{knd}{p_}_{nch}")
                            nc.tensor.matmul(
                                st["ps"][:, :],
                                w_sb[:, ct * OL + p_ * P: ct * OL + (p_ + 1) * P],
                                xt[ct][:, nch * 512:(nch + 1) * 512],
                                start=(ct == 0),
                                stop=(ct == NCT - 1),
                            )
                        return f

                    def mk_ev(st, nch):
                        def f():
                            nc.vector.tensor_copy(
                                dst[:, nch * 512:(nch + 1) * 512], st["ps"][:, :])
                        return f

                    for ct in range(NCT):
                        thunks.append(mk_mm(state, nch, ct))
                    thunks.append(mk_ev(state, nch))
                return thunks

            def v_proj():
                for nt in range(NNT):
                    ps = proj_ps.tile([P, 512], F32, tag="proj")
                    for ct in range(NCT):
                        nc.tensor.matmul(
                            ps[:, :],
                            xt[ct][:, nt * P:(nt + 1) * P],
                            wv_sb[:, ct * OL:(ct + 1) * OL],
                            start=(ct == 0),
                            stop=(ct == NCT - 1),
                        )
                    dst = v_sb[:, nt * VBLK:(nt + 1) * VBLK]
                    dst3 = dst.rearrange("p (h c) -> p h c", c=D + 1)[:, :, 0:D]
                    src3 = ps[:, :].rearrange("p (h c) -> p h c", c=D)
                    nc.vector.tensor_copy(dst3, src3)

            def attention(hp, fillers):
                """One head-pair phase. Returns epilogue thunks (emit later)."""
                hA = 2 * hp
                l_all = lp.tile([97, 1024], F32, tag="l_all", name=f"l_all{hp}")
                nc.any.memset(l_all[:, :], 1.0)
                fi = [0]
                slot = [0]

                def fill():
                    slot[0] += 1
                    if fi[0] < len(fillers):
                        t_ = fillers[fi[0]]
                        if isinstance(t_, tuple):
                            if slot[0] < t_[0]:
                                return
                            t_ = t_[1]
                        fillers[fi[0]] = None
                        fi[0] += 1
                        t_()

                def pv_pair(pvA, pvB, ex, jt):
                    for pv, h, off in ((pvA, hA, 0), (pvB, hA + 1, 512)):
                        nc.tensor.matmul(
                            pv[:, :],
                            v_sb[:, jt * VBLK + h * (D + 1):
                                 jt * VBLK + (h + 1) * (D + 1)],
                            ex[:, off:off + 512],
                            start=(jt == 0), stop=(jt == NNT - 1),
                        )

                for ib in range(NNC):
                    pv2 = pv_ps.tile([D + 1, 1024], F32, tag="pv2", name=f"pv{hp}_{ib}")
                    pvA = pv2[:, 0:512]
                    pvB = pv2[:, 512:1024]
                    pend = []
                    for jt in range(NNT):
                        ps = s_ps.tile([P, 1024], F32, tag="ps")
                        nc.tensor.matmul(
                            ps[:, 0:512],
                            kt[hp][0:D, jt * P:(jt + 1) * P],
                            qt[hp][0:D, ib * 512:(ib + 1) * 512],
                            start=True, stop=True,
                        )
                        nc.tensor.matmul(
                            ps[:, 512:1024],
                            kt[hp][D:P, jt * P:(jt + 1) * P],
                            qt[hp][D:P, ib * 512:(ib + 1) * 512],
                            start=True, stop=True,
                        )
                        if len(pend) >= 2:
                            pv_pair(pvA, pvB, *pend.pop(0))
                        else:
                            fill()
                        ex = ep.tile([P, 1024], BF16, tag="ex")
                        nc.scalar.activation(
                            ex[:, :], ps[:, :],
                            mybir.ActivationFunctionType.Exp, scale=SCALE,
                        )
                        fill()
                        pend.append((ex, jt))
                    for pd in pend:
                        pv_pair(pvA, pvB, *pd)
                    # stash denominators + unnormalized outputs; psum frees here
                    r = RL_ROWS[ib]
                    nc.vector.tensor_copy(l_all[r:r + 1, :], pv2[D:D + 1, :])
                    nc.vector.tensor_copy(
                        ot_sb[hp][0:D, ib * 512:(ib + 1) * 512], pvA[0:D, :])
                    nc.vector.tensor_copy(
                        ot_sb[hp][D:P, ib * 512:(ib + 1) * 512], pvB[0:D, :])

                # run remaining fillers (if any)
                for t_ in fillers[fi[0]:]:
                    if isinstance(t_, tuple):
                        t_ = t_[1]
                    if t_ is not None:
                        t_()
                fi[0] = len(fillers)

                # batched 1/l = exp(-ln(l)) on ACT (fast; accuracy ~1e-4 ok)
                rl = lp.tile([97, 1024], BF16, tag="rl", name=f"rl{hp}")
                tmp = lp.tile([97, 1024], F32, tag="rlf", name=f"rlf{hp}")
                nc.scalar.activation(tmp[:, :], l_all[:, :],
                                     mybir.ActivationFunctionType.Ln)
                nc.scalar.activation(rl[:, :], tmp[:, :],
                                     mybir.ActivationFunctionType.Exp, scale=-1.0)
                epi = []
                for ib in range(NNC):
                    def one(ib=ib, r=RL_ROWS[ib]):
                        rb = proj_ps.tile([P, 512], F32, tag="proj",
                                          name=f"rb{hp}_{ib}")
                        nc.tensor.matmul(rb[0:D, :], ones_sb[r:r + 1, :],
                                         rl[r:r + 1, 0:512], start=True, stop=True,
                                         tile_position=(r, 0))
                        nc.tensor.matmul(rb[D:P, :], ones_sb[r:r + 1, :],
                                         rl[r:r + 1, 512:1024], start=True, stop=True,
                                         tile_position=(r, D))
                        oA = ot_sb[hp][0:D, ib * 512:(ib + 1) * 512]
                        nc.vector.tensor_mul(oA, oA, rb[0:D, :])
                        oB = ot_sb[hp][D:P, ib * 512:(ib + 1) * 512]
                        nc.vector.tensor_mul(oB, oB, rb[D:P, :])
                    epi.append(one)
                return epi

            # ---- schedule ----
            for t in qk_proj_thunks(wq_sb, qt[0], 0, "q"):
                t()
            for t in qk_proj_thunks(wk_sb, kt[0], 0, "k"):
                t()
            v_proj()
            epi = []
            for hp in range(NOT):
                fillers = []
                if hp + 1 < NOT:
                    fillers = qk_proj_thunks(wq_sb, qt[hp + 1], hp + 1, "q")
                    fillers += qk_proj_thunks(wk_sb, kt[hp + 1], hp + 1, "k")

                # previous phase's normalize thunks: gate on slot >= 40 so
                # the batched reciprocal (DVE) has certainly drained, and
                # place at projection group boundaries when fillers exist
                for i_, t_ in enumerate(reversed(epi)):
                    k_ = len(epi) - 1 - i_
                    gated = (14 + 9 * k_, epi[k_])
                    fillers.insert(min(18 + 9 * k_, len(fillers)), gated)
                epi = attention(hp, fillers)
            # ---- tail: per-chunk normalize then its output projection ----
            for nch in range(NNC):
                epi[nch]()
                for ot in range(NCT):
                    ps = proj_ps.tile([P, 512], F32, tag="proj")
                    for p_ in range(NOT):
                        nc.tensor.matmul(
                            ps[:, :],
                            wo_sb[:, p_ * C + ot * P: p_ * C + (ot + 1) * P],
                            ot_sb[p_][:, nch * 512:(nch + 1) * 512],
                            start=(p_ == 0),
                            stop=(p_ == NOT - 1),
                        )
                    st = op.tile([P, 512], BF16, tag="os")
                    nc.vector.tensor_copy(st[:, :], ps[:, :])
                    nc.sync.dma_start(
                        out=outT[ot * P:(ot + 1) * P, nch * 512:(nch + 1) * 512],
                        in_=st[:, :],
                    )

    _split_waits(nc)
    return nc


_NC = None


def _in_maps(x, wq, wk, wv, wo):
    bf = ml_dtypes.bfloat16
    x = np.asarray(x, np.float32)
    wq = np.asarray(wq, np.float32)
    wk = np.asarray(wk, np.float32)
    wv = np.asarray(wv, np.float32)
    wo = np.asarray(wo, np.float32)
    maps = []
    for core in range(8):
        b, g = core // 2, core % 2
        sl = slice(g * OL, (g + 1) * OL)
        maps.append({
            "xT": np.ascontiguousarray(x[b].T).astype(bf),
            "wqT": np.ascontiguousarray(wq[sl, :].T).astype(bf),
            "wkT": np.ascontiguousarray(wk[sl, :].T).astype(bf),
            "wvT": np.ascontiguousarray(wv[sl, :].T).astype(bf),
            "woT": np.ascontiguousarray(wo[:, sl].T).astype(bf),
        })
    return maps


def kernel(x, wq, wk, wv, wo, bo):
    global _NC
    if _NC is None:
        _NC = build()
    maps = _in_maps(x, wq, wk, wv, wo)
    res = run_bass_kernel_spmd(_NC, maps, core_ids=list(range(8)))
    bo = np.asarray(bo, np.float32)
    out = np.empty((B, N, C), np.float32)
    for b in range(B):
        acc = (res.results[2 * b]["outT"].astype(np.float32)
               + res.results[2 * b + 1]["outT"].astype(np.float32))
        out[b] = acc.T + bo
    return out


# revision 40
# speedup vs baseline: 1.1666x; 1.1666x over previous
"""Multi-head attention (B=4, N=2048, C=1024, H=16, D=64) on 8 TRN2 NeuronCores.

Sharding: data-parallel over batch (4) x tensor-parallel over head halves (2).
Core c handles batch b=c//2, head-group g=c%2 (8 heads). Each core computes a
partial output (its heads' contribution to the out-projection, pre-bias);
the host sums the two partials per batch and adds the bias.

Per-core device graph (matmul inputs bf16, psum f32):
  QT/KT = (wq/wk slice)@x^T in transposed layout [o_local=512, n=2048]
  V     = x @ wv^T in normal layout [n, o_local], with a ones-column per head
  scoresT[j,i] per head pair via row-packed K=64 matmuls
  expT = exp(SCALE * scoresT)  (ACT engine, one op per 2 psum banks)
  PV: psum[65, i] = [V_h | ones]^T @ expT -> rows 0-63 = O'^T, row 64 = l_i
      (software-pipelined one jt behind the scores matmuls)
  normalize: batched 1/l = exp(-ln(l)) on the ACT engine (rows {0,32,64,96}
      of an l-stash tile), partition-broadcast via K=1 bf16 outer products
      into one shared PSUM bank per head pair, in-place multiply on O^T
  out-proj: outT[o, n] = woT^T @ O^T (partial, bf16 out; host sums + bias)

PV matmuls run two jt behind the scores matmuls (software pipeline), and
projection matmuls for phase p+1 plus phase p-1's slot-gated epilogue are
interleaved into the attention stream ("fillers") so the strictly-FIFO PE
queue never sits behind a dependency-stalled instruction.
"""
import sys

sys.path.insert(0, "/opt/trn_rl_repo")
import numpy as np
import ml_dtypes
import bass_rust
import concourse.bass as bass
import concourse.mybir as mybir
import concourse.tile as tile
from concourse.bass_utils import run_bass_kernel_spmd

BF16 = mybir.dt.bfloat16
F32 = mybir.dt.float32
P = 128
B, N, C = 4, 2048, 1024
HG, D = 8, 64          # heads per core, head dim
OL = HG * D            # 512 local o dims
SCALE = D ** -0.5
NCT = C // P           # 8 contraction tiles
NOT = OL // P          # 4 o tiles (head pairs)
NNT = N // P           # 16 n tiles of 128
NNC = N // 512         # 4 n chunks of 512
VBLK = OL + HG         # 520: 8 heads x (64 V cols + 1 ones col)
RL_ROWS = (0, 32, 64, 96)  # legal matmul base partitions for the rl rhs


def _split_waits(nc, max_waits=1):
    """This walrus build rejects >1 sem-wait per instruction; split excess
    waits into preceding same-engine single-wait NOPs."""
    uid = [0]
    for f in nc.m.functions:
        for bb in f.blocks:
            out = []
            changed = False
            for inst in bb.instructions:
                si = inst.sync_info
                if si is not None and len(si.on_wait) > max_waits:
                    waits = list(si.on_wait)
                    for w in waits[:-max_waits]:
                        uid[0] += 1
                        nop = bass_rust.InstNoOp(name=f"I-waitsplit-{uid[0]}")
                        nop.engine = inst.engine
                        nop.sync_info = bass_rust.SyncInfo(on_wait=[w], on_update=[])
                        try:
                            nc.register_instruction(nop, overwrite=True)
                        except Exception:
                            pass
                        out.append(nop)
                    si.on_wait = waits[-max_waits:]
                    changed = True
                out.append(inst)
            if changed:
                bb.instructions = out


def build():
    nc = bass.Bass("TRN2", target_bir_lowering=False, debug=False)
    xT = nc.dram_tensor("xT", [C, N], BF16, kind="ExternalInput")
    wqT = nc.dram_tensor("wqT", [C, OL], BF16, kind="ExternalInput")
    wkT = nc.dram_tensor("wkT", [C, OL], BF16, kind="ExternalInput")
    wvT = nc.dram_tensor("wvT", [C, OL], BF16, kind="ExternalInput")
    woT = nc.dram_tensor("woT", [OL, C], BF16, kind="ExternalInput")
    outT = nc.dram_tensor("outT", [C, N], BF16, kind="ExternalOutput")

    with tile.TileContext(nc) as tc:
        with (
            tc.tile_pool(name="persist", bufs=1) as pp,
            tc.tile_pool(name="ex", bufs=4) as ep,
            tc.tile_pool(name="lrl", bufs=2) as lp,
            tc.tile_pool(name="ostage", bufs=8) as op,
            tc.tile_pool(name="proj_ps", bufs=2, space="PSUM") as proj_ps,
            tc.tile_pool(name="s_ps", bufs=2, space="PSUM") as s_ps,
            tc.tile_pool(name="pv_ps", bufs=1, space="PSUM") as pv_ps,
        ):
            # ---- persistent SBUF ----
            xt = [pp.tile([P, N], BF16, name=f"xt{i}", tag=f"xt{i}") for i in range(NCT)]
            wq_sb = pp.tile([P, NCT * OL], BF16, tag="wq")
            wk_sb = pp.tile([P, NCT * OL], BF16, tag="wk")
            wv_sb = pp.tile([P, NCT * OL], BF16, tag="wv")
            wo_sb = pp.tile([P, NOT * C], BF16, tag="wo")
            qt = [pp.tile([P, N], BF16, name=f"qt{p}", tag=f"qt{p}") for p in range(NOT)]
            kt = [pp.tile([P, N], BF16, name=f"kt{p}", tag=f"kt{p}") for p in range(NOT)]
            v_sb = pp.tile([P, NNT * VBLK], BF16, tag="v")
            ot_sb = [pp.tile([P, N], BF16, name=f"ot{p}", tag=f"ot{p}") for p in range(NOT)]
            ones_sb = pp.tile([97, D], BF16, tag="ones_sb")

            # ---- loads ----
            for i in range(NCT):
                nc.sync.dma_start(
                    out=wq_sb[:, i * OL:(i + 1) * OL],
                    in_=wqT[i * P:(i + 1) * P, :],
                )
                nc.sync.dma_start(
                    out=xt[i][:, 0:512], in_=xT[i * P:(i + 1) * P, 0:512])
            for i in range(NCT):
                nc.sync.dma_start(
                    out=wk_sb[:, i * OL:(i + 1) * OL],
                    in_=wkT[i * P:(i + 1) * P, :],
                )
            for nch in range(1, NNC):
                for i in range(NCT):
                    nc.sync.dma_start(
                        out=xt[i][:, nch * 512:(nch + 1) * 512],
                        in_=xT[i * P:(i + 1) * P, nch * 512:(nch + 1) * 512],
                    )
            for i in range(NCT):
                nc.sync.dma_start(
                    out=wv_sb[:, i * OL:(i + 1) * OL],
                    in_=wvT[i * P:(i + 1) * P, :],
                )
            for p_ in range(NOT):
                nc.sync.dma_start(
                    out=wo_sb[:, p_ * C:(p_ + 1) * C],
                    in_=woT[p_ * P:(p_ + 1) * P, :],
                )
            # ones columns for the softmax-denominator rows
            nc.any.memset(v_sb[:, :], 1.0)
            nc.any.memset(ones_sb[:, :], 1.0)

            def qk_proj_thunks(w_sb, dst, p_, knd):
                """Emit-callables for dst = transposed projection (Q or K)."""
                thunks = []
                for nch in range(NNC):
                    state = {}

                    def mk_mm(st, nch, ct):
                        def f():
                            if "ps" not in st:
                                st["ps"] = proj_ps.tile(
                                    [P, 512], F32, tag="proj",
                                    name=f"prj_{knd}{p_}_{nch}")
                            nc.tensor.matmul(
                                st["ps"][:, :],
                                w_sb[:, ct * OL + p_ * P: ct * OL + (p_ + 1) * P],
                                xt[ct][:, nch * 512:(nch + 1) * 512],
                                start=(ct == 0),
                                stop=(ct == NCT - 1),
                            )
                        return f

                    def mk_ev(st, nch):
                        def f():
                            nc.vector.tensor_copy(
                                dst[:, nch * 512:(nch + 1) * 512], st["ps"][:, :])
                        return f

                    for ct in range(NCT):
                        thunks.append(mk_mm(state, nch, ct))
                    thunks.append(mk_ev(state, nch))
                return thunks

            def v_proj():
                for nt in range(NNT):
                    ps = proj_ps.tile([P, 512], F32, tag="proj")
                    for ct in range(NCT):
                        nc.tensor.matmul(
                            ps[:, :],
                            xt[ct][:, nt * P:(nt + 1) * P],
                            wv_sb[:, ct * OL:(ct + 1) * OL],
                            start=(ct == 0),
                            stop=(ct == NCT - 1),
                        )
                    dst = v_sb[:, nt * VBLK:(nt + 1) * VBLK]
                    dst3 = dst.rearrange("p (h c) -> p h c", c=D + 1)[:, :, 0:D]
                    src3 = ps[:, :].rearrange("p (h c) -> p h c", c=D)
                    nc.vector.tensor_copy(dst3, src3)

            def attention(hp, fillers):
                """One head-pair phase. Returns epilogue thunks (emit later)."""
                hA = 2 * hp
                l_all = lp.tile([97, 1024], F32, tag="l_all", name=f"l_all{hp}")
                nc.any.memset(l_all[:, :], 1.0)
                fi = [0]
                slot = [0]

                def fill():
                    slot[0] += 1
                    if fi[0] < len(fillers):
                        t_ = fillers[fi[0]]
                        if isinstance(t_, tuple):
                            if slot[0] < t_[0]:
                                return
                            t_ = t_[1]
                        fillers[fi[0]] = None
                        fi[0] += 1
                        t_()

                def pv_pair(pvA, pvB, ex, jt):
                    for pv, h, off in ((pvA, hA, 0), (pvB, hA + 1, 512)):
                        nc.tensor.matmul(
                            pv[:, :],
                            v_sb[:, jt * VBLK + h * (D + 1):
                                 jt * VBLK + (h + 1) * (D + 1)],
                            ex[:, off:off + 512],
                            start=(jt == 0), stop=(jt == NNT - 1),
                        )

                for ib in range(NNC):
                    pv2 = pv_ps.tile([D + 1, 1024], F32, tag="pv2", name=f"pv{hp}_{ib}")
                    pvA = pv2[:, 0:512]
                    pvB = pv2[:, 512:1024]
                    pend = []
                    for jt in range(NNT):
                        ps = s_ps.tile([P, 1024], F32, tag="ps")
                        nc.tensor.matmul(
                            ps[:, 0:512],
                            kt[hp][0:D, jt * P:(jt + 1) * P],
                            qt[hp][0:D, ib * 512:(ib + 1) * 512],
                            start=True, stop=True,
                        )
                        nc.tensor.matmul(
                            ps[:, 512:1024],
                            kt[hp][D:P, jt * P:(jt + 1) * P],
                            qt[hp][D:P, ib * 512:(ib + 1) * 512],
                            start=True, stop=True,
                        )
                        if len(pend) >= 2:
                            pv_pair(pvA, pvB, *pend.pop(0))
                            fill()
                        else:
                            fill()
                        ex = ep.tile([P, 1024], BF16, tag="ex")
                        nc.scalar.activation(
                            ex[:, :], ps[:, :],
                            mybir.ActivationFunctionType.Exp, scale=SCALE,
                        )
                        fill()
                        pend.append((ex, jt))
                    for pd in pend:
                        pv_pair(pvA, pvB, *pd)
                    # stash denominators + unnormalized outputs; psum frees here
                    r = RL_ROWS[ib]
                    nc.vector.tensor_copy(l_all[r:r + 1, :], pv2[D:D + 1, :])
                    nc.vector.tensor_copy(
                        ot_sb[hp][0:D, ib * 512:(ib + 1) * 512], pvA[0:D, :])
                    nc.vector.tensor_copy(
                        ot_sb[hp][D:P, ib * 512:(ib + 1) * 512], pvB[0:D, :])

                # run remaining fillers (if any)
                for t_ in fillers[fi[0]:]:
                    if isinstance(t_, tuple):
                        t_ = t_[1]
                    if t_ is not None:
                        t_()
                fi[0] = len(fillers)

                # batched 1/l = exp(-ln(l)) on ACT (fast; accuracy ~1e-4 ok)
                rl = lp.tile([97, 1024], BF16, tag="rl", name=f"rl{hp}")
                tmp = lp.tile([97, 1024], F32, tag="rlf", name=f"rlf{hp}")
                nc.scalar.activation(tmp[:, :], l_all[:, :],
                                     mybir.ActivationFunctionType.Ln)
                nc.scalar.activation(rl[:, :], tmp[:, :],
                                     mybir.ActivationFunctionType.Exp, scale=-1.0)
                epi = []
                for ib in range(NNC):
                    def one(ib=ib, r=RL_ROWS[ib]):
                        rb = proj_ps.tile([P, 512], F32, tag="proj",
                                          name=f"rb{hp}_{ib}")
                        nc.tensor.matmul(rb[0:D, :], ones_sb[r:r + 1, :],
                                         rl[r:r + 1, 0:512], start=True, stop=True,
                                         tile_position=(r, 0))
                        nc.tensor.matmul(rb[D:P, :], ones_sb[r:r + 1, :],
                                         rl[r:r + 1, 512:1024], start=True, stop=True,
                                         tile_position=(r, D))
                        oA = ot_sb[hp][0:D, ib * 512:(ib + 1) * 512]
                        nc.vector.tensor_mul(oA, oA, rb[0:D, :])
                        oB = ot_sb[hp][D:P, ib * 512:(ib + 1) * 512]
                        nc.vector.tensor_mul(oB, oB, rb[D:P, :])
                    epi.append(one)
                return epi

            # ---- schedule ----
            for t in qk_proj_thunks(wq_sb, qt[0], 0, "q"):
                t()
            for t in qk_proj_thunks(wk_sb, kt[0], 0, "k"):
                t()
            v_proj()
            epi = []
            for hp in range(NOT):
                fillers = []
                if hp + 1 < NOT:
                    fillers = qk_proj_thunks(wq_sb, qt[hp + 1], hp + 1, "q")
                    fillers += qk_proj_thunks(wk_sb, kt[hp + 1], hp + 1, "k")

                # previous phase's normalize thunks: gate on slot >= 40 so
                # the batched reciprocal (DVE) has certainly drained, and
                # place at projection group boundaries when fillers exist
                for i_, t_ in enumerate(reversed(epi)):
                    k_ = len(epi) - 1 - i_
                    gated = (14 + 9 * k_, epi[k_])
                    fillers.insert(min(18 + 9 * k_, len(fillers)), gated)
                epi = attention(hp, fillers)
            # ---- tail: per-chunk normalize then its output projection ----
            for nch in range(NNC):
                epi[nch]()
                for ot in range(NCT):
                    ps = proj_ps.tile([P, 512], F32, tag="proj")
                    for p_ in range(NOT):
                        nc.tensor.matmul(
                            ps[:, :],
                            wo_sb[:, p_ * C + ot * P: p_ * C + (ot + 1) * P],
                            ot_sb[p_][:, nch * 512:(nch + 1) * 512],
                            start=(p_ == 0),
                            stop=(p_ == NOT - 1),
                        )
                    st = op.tile([P, 512], BF16, tag="os")
                    nc.vector.tensor_copy(st[:, :], ps[:, :])
                    nc.sync.dma_start(
                        out=outT[ot * P:(ot + 1) * P, nch * 512:(nch + 1) * 512],
                        in_=st[:, :],
                    )

    _split_waits(nc)
    return nc


_NC = None


def _in_maps(x, wq, wk, wv, wo):
    bf = ml_dtypes.bfloat16
    x = np.asarray(x, np.float32)
    wq = np.asarray(wq, np.float32)
    wk = np.asarray(wk, np.float32)
    wv = np.asarray(wv, np.float32)
    wo = np.asarray(wo, np.float32)
    maps = []
    for core in range(8):
        b, g = core // 2, core % 2
        sl = slice(g * OL, (g + 1) * OL)
        maps.append({
            "xT": np.ascontiguousarray(x[b].T).astype(bf),
            "wqT": np.ascontiguousarray(wq[sl, :].T).astype(bf),
            "wkT": np.ascontiguousarray(wk[sl, :].T).astype(bf),
            "wvT": np.ascontiguousarray(wv[sl, :].T).astype(bf),
            "woT": np.ascontiguousarray(wo[:, sl].T).astype(bf),
        })
    return maps


def kernel(x, wq, wk, wv, wo, bo):
    global _NC
    if _NC is None:
        _NC = build()
    maps = _in_maps(x, wq, wk, wv, wo)
    res = run_bass_kernel_spmd(_NC, maps, core_ids=list(range(8)))
    bo = np.asarray(bo, np.float32)
    out = np.empty((B, N, C), np.float32)
    for b in range(B):
        acc = (res.results[2 * b]["outT"].astype(np.float32)
               + res.results[2 * b + 1]["outT"].astype(np.float32))
        out[b] = acc.T + bo
    return out


# revision 41
# speedup vs baseline: 1.1975x; 1.0265x over previous
"""Multi-head attention (B=4, N=2048, C=1024, H=16, D=64) on 8 TRN2 NeuronCores.

Sharding: data-parallel over batch (4) x tensor-parallel over head halves (2).
Core c handles batch b=c//2, head-group g=c%2 (8 heads). Each core computes a
partial output (its heads' contribution to the out-projection, pre-bias);
the host sums the two partials per batch and adds the bias.

Per-core device graph (matmul inputs bf16, psum f32):
  QT/KT = (wq/wk slice)@x^T in transposed layout [o_local=512, n=2048]
  V     = x @ wv^T in normal layout [n, o_local], with a ones-column per head
  scoresT[j,i] per head pair via row-packed K=64 matmuls
  expT = exp(SCALE * scoresT)  (ACT engine, one op per 2 psum banks)
  PV: psum[65, i] = [V_h | ones]^T @ expT -> rows 0-63 = O'^T, row 64 = l_i
      (software-pipelined one jt behind the scores matmuls)
  normalize: batched 1/l = exp(-ln(l)) on the ACT engine (rows {0,32,64,96}
      of an l-stash tile), partition-broadcast via K=1 bf16 outer products
      into one shared PSUM bank per head pair, in-place multiply on O^T
  out-proj: outT[o, n] = woT^T @ O^T (partial, bf16 out; host sums + bias)

PV matmuls run two jt behind the scores matmuls (software pipeline), and
projection matmuls for phase p+1 plus phase p-1's slot-gated epilogue are
interleaved into the attention stream ("fillers") so the strictly-FIFO PE
queue never sits behind a dependency-stalled instruction.
"""
import sys

sys.path.insert(0, "/opt/trn_rl_repo")
import numpy as np
import ml_dtypes
import bass_rust
import concourse.bass as bass
import concourse.mybir as mybir
import concourse.tile as tile
from concourse.bass_utils import run_bass_kernel_spmd

BF16 = mybir.dt.bfloat16
F32 = mybir.dt.float32
P = 128
B, N, C = 4, 2048, 1024
HG, D = 8, 64          # heads per core, head dim
OL = HG * D            # 512 local o dims
SCALE = D ** -0.5
NCT = C // P           # 8 contraction tiles
NOT = OL // P          # 4 o tiles (head pairs)
NNT = N // P           # 16 n tiles of 128
NNC = N // 512         # 4 n chunks of 512
VBLK = OL + HG         # 520: 8 heads x (64 V cols + 1 ones col)
RL_ROWS = (0, 32, 64, 96)  # legal matmul base partitions for the rl rhs


def _split_waits(nc, max_waits=1):
    """This walrus build rejects >1 sem-wait per instruction; split excess
    waits into preceding same-engine single-wait NOPs."""
    uid = [0]
    for f in nc.m.functions:
        for bb in f.blocks:
            out = []
            changed = False
            for inst in bb.instructions:
                si = inst.sync_info
                if si is not None and len(si.on_wait) > max_waits:
                    waits = list(si.on_wait)
                    for w in waits[:-max_waits]:
                        uid[0] += 1
                        nop = bass_rust.InstNoOp(name=f"I-waitsplit-{uid[0]}")
                        nop.engine = inst.engine
                        nop.sync_info = bass_rust.SyncInfo(on_wait=[w], on_update=[])
                        try:
                            nc.register_instruction(nop, overwrite=True)
                        except Exception:
                            pass
                        out.append(nop)
                    si.on_wait = waits[-max_waits:]
                    changed = True
                out.append(inst)
            if changed:
                bb.instructions = out


def build():
    nc = bass.Bass("TRN2", target_bir_lowering=False, debug=False)
    xT = nc.dram_tensor("xT", [C, N], BF16, kind="ExternalInput")
    wqT = nc.dram_tensor("wqT", [C, OL], BF16, kind="ExternalInput")
    wkT = nc.dram_tensor("wkT", [C, OL], BF16, kind="ExternalInput")
    wvT = nc.dram_tensor("wvT", [C, OL], BF16, kind="ExternalInput")
    woT = nc.dram_tensor("woT", [OL, C], BF16, kind="ExternalInput")
    outT = nc.dram_tensor("outT", [C, N], BF16, kind="ExternalOutput")

    with tile.TileContext(nc) as tc:
        with (
            tc.tile_pool(name="persist", bufs=1) as pp,
            tc.tile_pool(name="ex", bufs=4) as ep,
            tc.tile_pool(name="lrl", bufs=2) as lp,
            tc.tile_pool(name="ostage", bufs=8) as op,
            tc.tile_pool(name="proj_ps", bufs=2, space="PSUM") as proj_ps,
            tc.tile_pool(name="s_ps", bufs=2, space="PSUM") as s_ps,
            tc.tile_pool(name="pv_ps", bufs=1, space="PSUM") as pv_ps,
        ):
            # ---- persistent SBUF ----
            xt = [pp.tile([P, N], BF16, name=f"xt{i}", tag=f"xt{i}") for i in range(NCT)]
            wq_sb = pp.tile([P, NCT * OL], BF16, tag="wq")
            wk_sb = pp.tile([P, NCT * OL], BF16, tag="wk")
            wv_sb = pp.tile([P, NCT * OL], BF16, tag="wv")
            wo_sb = pp.tile([P, NOT * C], BF16, tag="wo")
            qt = [pp.tile([P, N], BF16, name=f"qt{p}", tag=f"qt{p}") for p in range(NOT)]
            kt = [pp.tile([P, N], BF16, name=f"kt{p}", tag=f"kt{p}") for p in range(NOT)]
            v_sb = pp.tile([P, NNT * VBLK], BF16, tag="v")
            ot_sb = [pp.tile([P, N], BF16, name=f"ot{p}", tag=f"ot{p}") for p in range(NOT)]
            ones_sb = pp.tile([97, D], BF16, tag="ones_sb")

            # ---- loads ----
            for i in range(NCT):
                nc.sync.dma_start(
                    out=wq_sb[:, i * OL:(i + 1) * OL],
                    in_=wqT[i * P:(i + 1) * P, :],
                )
                nc.sync.dma_start(
                    out=xt[i][:, 0:512], in_=xT[i * P:(i + 1) * P, 0:512])
            for i in range(NCT):
                nc.sync.dma_start(
                    out=wk_sb[:, i * OL:(i + 1) * OL],
                    in_=wkT[i * P:(i + 1) * P, :],
                )
            for nch in range(1, NNC):
                for i in range(NCT):
                    nc.sync.dma_start(
                        out=xt[i][:, nch * 512:(nch + 1) * 512],
                        in_=xT[i * P:(i + 1) * P, nch * 512:(nch + 1) * 512],
                    )
            for i in range(NCT):
                nc.sync.dma_start(
                    out=wv_sb[:, i * OL:(i + 1) * OL],
                    in_=wvT[i * P:(i + 1) * P, :],
                )
            for p_ in range(NOT):
                nc.sync.dma_start(
                    out=wo_sb[:, p_ * C:(p_ + 1) * C],
                    in_=woT[p_ * P:(p_ + 1) * P, :],
                )
            # ones columns for the softmax-denominator rows
            nc.any.memset(v_sb[:, :], 1.0)
            nc.any.memset(ones_sb[:, :], 1.0)

            def qk_proj_thunks(w_sb, dst, p_, knd):
                """Emit-callables for dst = transposed projection (Q or K)."""
                thunks = []
                for nch in range(NNC):
                    state = {}

                    def mk_mm(st, nch, ct):
                        def f():
                            if "ps" not in st:
                                st["ps"] = proj_ps.tile(
                                    [P, 512], F32, tag="proj",
                                    name=f"prj_{knd}{p_}_{nch}")
                            nc.tensor.matmul(
                                st["ps"][:, :],
                                w_sb[:, ct * OL + p_ * P: ct * OL + (p_ + 1) * P],
                                xt[ct][:, nch * 512:(nch + 1) * 512],
                                start=(ct == 0),
                                stop=(ct == NCT - 1),
                            )
                        return f

                    def mk_ev(st, nch):
                        def f():
                            nc.vector.tensor_copy(
                                dst[:, nch * 512:(nch + 1) * 512], st["ps"][:, :])
                        return f

                    for ct in range(NCT):
                        thunks.append(mk_mm(state, nch, ct))
                    thunks.append(mk_ev(state, nch))
                return thunks

            def v_proj():
                for nt in range(NNT):
                    ps = proj_ps.tile([P, 512], F32, tag="proj")
                    for ct in range(NCT):
                        nc.tensor.matmul(
                            ps[:, :],
                            xt[ct][:, nt * P:(nt + 1) * P],
                            wv_sb[:, ct * OL:(ct + 1) * OL],
                            start=(ct == 0),
                            stop=(ct == NCT - 1),
                        )
                    dst = v_sb[:, nt * VBLK:(nt + 1) * VBLK]
                    dst3 = dst.rearrange("p (h c) -> p h c", c=D + 1)[:, :, 0:D]
                    src3 = ps[:, :].rearrange("p (h c) -> p h c", c=D)
                    nc.vector.tensor_copy(dst3, src3)

            def attention(hp, fillers):
                """One head-pair phase. Returns epilogue thunks (emit later)."""
                hA = 2 * hp
                l_all = lp.tile([97, 1024], F32, tag="l_all", name=f"l_all{hp}")
                nc.any.memset(l_all[:, :], 1.0)
                fi = [0]
                slot = [0]

                def fill():
                    slot[0] += 1
                    if fi[0] < len(fillers):
                        t_ = fillers[fi[0]]
                        if isinstance(t_, tuple):
                            if slot[0] < t_[0]:
                                return
                            t_ = t_[1]
                        fillers[fi[0]] = None
                        fi[0] += 1
                        t_()

                def pv_pair(pvA, pvB, ex, jt):
                    for pv, h, off in ((pvA, hA, 0), (pvB, hA + 1, 512)):
                        nc.tensor.matmul(
                            pv[:, :],
                            v_sb[:, jt * VBLK + h * (D + 1):
                                 jt * VBLK + (h + 1) * (D + 1)],
                            ex[:, off:off + 512],
                            start=(jt == 0), stop=(jt == NNT - 1),
                        )

                for ib in range(NNC):
                    pv2 = pv_ps.tile([D + 1, 1024], F32, tag="pv2", name=f"pv{hp}_{ib}")
                    pvA = pv2[:, 0:512]
                    pvB = pv2[:, 512:1024]
                    pend = []
                    for jt in range(NNT):
                        ps = s_ps.tile([P, 1024], F32, tag="ps")
                        nc.tensor.matmul(
                            ps[:, 0:512],
                            kt[hp][0:D, jt * P:(jt + 1) * P],
                            qt[hp][0:D, ib * 512:(ib + 1) * 512],
                            start=True, stop=True,
                        )
                        nc.tensor.matmul(
                            ps[:, 512:1024],
                            kt[hp][D:P, jt * P:(jt + 1) * P],
                            qt[hp][D:P, ib * 512:(ib + 1) * 512],
                            start=True, stop=True,
                        )
                        if len(pend) >= 2:
                            pv_pair(pvA, pvB, *pend.pop(0))
                        else:
                            fill()
                        ex = ep.tile([P, 1024], BF16, tag="ex")
                        nc.scalar.activation(
                            ex[:, :], ps[:, :],
                            mybir.ActivationFunctionType.Exp, scale=SCALE,
                        )
                        fill()
                        pend.append((ex, jt))
                    for pd in pend:
                        pv_pair(pvA, pvB, *pd)
                    # stash denominators + unnormalized outputs; psum frees here
                    r = RL_ROWS[ib]
                    nc.vector.tensor_copy(l_all[r:r + 1, :], pv2[D:D + 1, :])
                    nc.vector.tensor_copy(
                        ot_sb[hp][0:D, ib * 512:(ib + 1) * 512], pvA[0:D, :])
                    nc.vector.tensor_copy(
                        ot_sb[hp][D:P, ib * 512:(ib + 1) * 512], pvB[0:D, :])

                # run remaining fillers (if any)
                for t_ in fillers[fi[0]:]:
                    if isinstance(t_, tuple):
                        t_ = t_[1]
                    if t_ is not None:
                        t_()
                fi[0] = len(fillers)

                # batched 1/l = exp(-ln(l)) on ACT (fast; accuracy ~1e-4 ok)
                rl = lp.tile([97, 1024], BF16, tag="rl", name=f"rl{hp}")
                tmp = lp.tile([97, 1024], F32, tag="rlf", name=f"rlf{hp}")
                nc.scalar.activation(tmp[:, :], l_all[:, :],
                                     mybir.ActivationFunctionType.Ln)
                nc.scalar.activation(rl[:, :], tmp[:, :],
                                     mybir.ActivationFunctionType.Exp, scale=-1.0)
                epi = []
                for ib in range(NNC):
                    def one(ib=ib, r=RL_ROWS[ib]):
                        rb = proj_ps.tile([P, 512], F32, tag="proj",
                                          name=f"rb{hp}_{ib}")
                        nc.tensor.matmul(rb[0:D, :], ones_sb[r:r + 1, :],
                                         rl[r:r + 1, 0:512], start=True, stop=True,
                                         tile_position=(r, 0))
                        nc.tensor.matmul(rb[D:P, :], ones_sb[r:r + 1, :],
                                         rl[r:r + 1, 512:1024], start=True, stop=True,
                                         tile_position=(r, D))
                        oA = ot_sb[hp][0:D, ib * 512:(ib + 1) * 512]
                        nc.vector.tensor_mul(oA, oA, rb[0:D, :])
                        oB = ot_sb[hp][D:P, ib * 512:(ib + 1) * 512]
                        nc.vector.tensor_mul(oB, oB, rb[D:P, :])
                    epi.append(one)
                return epi

            # ---- schedule ----
            for t in qk_proj_thunks(wq_sb, qt[0], 0, "q"):
                t()
            for t in qk_proj_thunks(wk_sb, kt[0], 0, "k"):
                t()
            v_proj()
            epi = []
            for hp in range(NOT):
                fillers = []
                if hp + 1 < NOT:
                    fillers = qk_proj_thunks(wq_sb, qt[hp + 1], hp + 1, "q")
                    fillers += qk_proj_thunks(wk_sb, kt[hp + 1], hp + 1, "k")

                # previous phase's normalize thunks: gate on slot >= 40 so
                # the batched reciprocal (DVE) has certainly drained, and
                # place at projection group boundaries when fillers exist
                for i_, t_ in enumerate(reversed(epi)):
                    k_ = len(epi) - 1 - i_
                    gated = (14 + 9 * k_, epi[k_])
                    fillers.insert(min(18 + 9 * k_, len(fillers)), gated)
                epi = attention(hp, fillers)
            # ---- tail: per-chunk normalize then its output projection ----
            for nch in range(NNC):
                epi[nch]()
                for ot in range(NCT):
                    ps = proj_ps.tile([P, 512], F32, tag="proj")
                    for p_ in range(NOT):
                        nc.tensor.matmul(
                            ps[:, :],
                            wo_sb[:, p_ * C + ot * P: p_ * C + (ot + 1) * P],
                            ot_sb[p_][:, nch * 512:(nch + 1) * 512],
                            start=(p_ == 0),
                            stop=(p_ == NOT - 1),
                        )
                    st = op.tile([P, 512], BF16, tag="os")
                    nc.vector.tensor_copy(st[:, :], ps[:, :])
                    nc.sync.dma_start(
                        out=outT[ot * P:(ot + 1) * P, nch * 512:(nch + 1) * 512],
                        in_=st[:, :],
                    )

    _split_waits(nc)
    return nc


_NC = None


def _in_maps(x, wq, wk, wv, wo):
    bf = ml_dtypes.bfloat16
    x = np.asarray(x, np.float32)
    wq = np.asarray(wq, np.float32)
    wk = np.asarray(wk, np.float32)
    wv = np.asarray(wv, np.float32)
    wo = np.asarray(wo, np.float32)
    maps = []
    for core in range(8):
        b, g = core // 2, core % 2
        sl = slice(g * OL, (g + 1) * OL)
        maps.append({
            "xT": np.ascontiguousarray(x[b].T).astype(bf),
            "wqT": np.ascontiguousarray(wq[sl, :].T).astype(bf),
            "wkT": np.ascontiguousarray(wk[sl, :].T).astype(bf),
            "wvT": np.ascontiguousarray(wv[sl, :].T).astype(bf),
            "woT": np.ascontiguousarray(wo[:, sl].T).astype(bf),
        })
    return maps


def kernel(x, wq, wk, wv, wo, bo):
    global _NC
    if _NC is None:
        _NC = build()
    maps = _in_maps(x, wq, wk, wv, wo)
    res = run_bass_kernel_spmd(_NC, maps, core_ids=list(range(8)))
    bo = np.asarray(bo, np.float32)
    out = np.empty((B, N, C), np.float32)
    for b in range(B):
        acc = (res.results[2 * b]["outT"].astype(np.float32)
               + res.results[2 * b + 1]["outT"].astype(np.float32))
        out[b] = acc.T + bo
    return out
